# revision 30
# baseline (speedup 1.0000x reference)
"""AdaptiveRouter (MoE routing) Trainium2 kernel — 8 NeuronCores, data-parallel.

Reference computation (per problem):
    logits  = hidden @ router_weight.T + log(softmax(importance) + eps), / temperature
    top2    -> indices + softmax over the 2 selected logits
    probs   = softmax(logits); expert_load = probs.mean(0);
    load_variance = var(expert_load, ddof=1); entropy = -(p*log(p+eps)).sum(-1).mean()

Sharding: tokens are split 8x (2048/core). router weight / importance /
temperature replicated. Global stats are assembled on host from tiny
per-core partial sums (no collectives needed).

Device-side numerics: hidden and router weight are re-encoded on host as
bf16 (hi, lo) pairs — hi + lo == x to ~2^-18 relative; total DMA bytes are
unchanged (2+2 vs 4 per element). The two weight halves are packed as
columns 0:64 / 64:128 of one 128-wide stationary operand, so each moving
operand (hi, lo) streams through the PE once, producing wh- and wl-partial
sums in PSUM partitions 0:64 / 64:128; all four split products accumulate.
The halves are summed token-major after the PE transpose. bf16 streams
1 cyc/row vs fp32's effective 4.

The hidden shard is pre-transposed on host ([H, tokens], contraction on
SBUF partitions) and laid out pass-major with 4 contraction chunks per DMA
so every hidden DMA is a single large fully-contiguous read.
"""

import sys
import numpy as np

sys.path.insert(0, "/opt/trn_rl_repo")

from contextlib import ExitStack

import concourse.bass as bass
import concourse.bacc as bacc
import concourse.mybir as mybir
import concourse.tile as tile
import concourse.masks as masks
from concourse.bass_utils import run_bass_kernel_spmd

F32 = mybir.dt.float32
BF16 = mybir.dt.bfloat16
U32 = mybir.dt.uint32
AF = mybir.ActivationFunctionType
ALU = mybir.AluOpType
AX = mybir.AxisListType

# Problem geometry (hardcoded per spec nn_AdaptiveRouter_50534585205486)
N, H, E = 16384, 4096, 64
NCORES = 8
NT = N // NCORES            # tokens per core (2048)
HC = H // 128               # contraction chunks (32)
PAIR = 2                    # contraction chunks per hidden DMA (hi+lo merged)
EPS = 1e-8
TOPK = 2
# uneven passes: the last pass is small so the final (serial) epilogue is short
PASS_TOK = [1536, 512]
PASS_OFF = [0, 1536]


def build_nc():
    nc = bacc.Bacc("TRN2", target_bir_lowering=False, debug=False)

    # hidden hi+lo merged, one rectangular tensor per pass: each DMA block is
    # [128, 2*PAIR*tp] = [hi chunk pair | lo chunk pair], fully contiguous
    hts = []
    for p, tp in enumerate(PASS_TOK):
        hil = nc.dram_tensor(f"hil{p}", [HC // PAIR * 128, 2 * PAIR * tp], BF16,
                             kind="ExternalInput")
        hts.append(hil.rearrange("(c q) u -> c q u", q=128))
    # packed per-chunk stationary: columns 0:E = wh, E:2E = wl
    wtp = nc.dram_tensor("wtp", [128, HC * 2 * E], BF16, kind="ExternalInput")
    imp = nc.dram_tensor("imp", [1, E], F32, kind="ExternalInput")
    temp = nc.dram_tensor("temp", [1, 1], F32, kind="ExternalInput")

    out0 = nc.dram_tensor("out0", [NT, E], F32, kind="ExternalOutput")
    # wi rows are ordered (partition, block) within each pass's token range
    out1 = nc.dram_tensor("out1", [NT, 4], F32, kind="ExternalOutput")
    pacc_d = nc.dram_tensor("pacc", [128, E], F32, kind="ExternalOutput")
    eacc_d = nc.dram_tensor("eacc", [128, 1], F32, kind="ExternalOutput")

    with ExitStack() as ctx:
        tc = ctx.enter_context(tile.TileContext(nc))
        cpool = ctx.enter_context(tc.tile_pool(name="const", bufs=1))
        hipool = ctx.enter_context(tc.tile_pool(name="hil", bufs=3))
        acsbpool = ctx.enter_context(tc.tile_pool(name="acsb", bufs=2))
        tmpool = ctx.enter_context(tc.tile_pool(name="tm", bufs=2))
        spool = ctx.enter_context(tc.tile_pool(name="scratch", bufs=2))
        accpool = ctx.enter_context(tc.tile_pool(name="acc", bufs=1))
        ps_acc = ctx.enter_context(
            tc.tile_pool(name="psacc", bufs=1, space=bass.MemorySpace.PSUM)
        )
        ps_t = ctx.enter_context(
            tc.tile_pool(name="pst", bufs=2, space=bass.MemorySpace.PSUM)
        )

        # ---- weights first: the first matmul gates on these ------------
        wp = cpool.tile([128, HC, 2 * E], BF16)
        nc.scalar.dma_start(wp[:], wtp[:, :])

        timp = cpool.tile([1, E], F32)
        nc.sync.dma_start(timp[:], imp[:, :])
        ttemp = cpool.tile([1, 1], F32)
        nc.sync.dma_start(ttemp[:], temp[:, :])

        ident = cpool.tile([128, 128], F32)
        masks.make_identity(nc, ident[:])

        ones_row = cpool.tile([1, 128], F32)
        nc.vector.memset(ones_row[:], 1.0)
        eps1 = cpool.tile([1, 1], F32)
        nc.vector.memset(eps1[:], EPS)
        eps128 = cpool.tile([128, 1], F32)
        nc.vector.memset(eps128[:], EPS)
        zero128 = cpool.tile([128, 1], F32)
        nc.vector.memset(zero128[:], 0.0)

        # log(softmax(importance) + eps) on partition 0, scaled by 1/temp
        nm = cpool.tile([1, 1], F32)
        nc.vector.reduce_max(nm[:], timp[:], axis=AX.X, negate=True)
        te = cpool.tile([1, E], F32)
        nc.scalar.activation(te[:], timp[:], AF.Exp, bias=nm[:])
        tsum = cpool.tile([1, 1], F32)
        nc.vector.reduce_sum(tsum[:], te[:], axis=AX.X)
        trcp = cpool.tile([1, 1], F32)
        nc.vector.reciprocal(trcp[:], tsum[:])
        smx = cpool.tile([1, E], F32)
        nc.vector.tensor_scalar_mul(smx[:], te[:], trcp[:])
        lbrow = cpool.tile([1, E], F32)
        nc.scalar.activation(lbrow[:], smx[:], AF.Ln, bias=eps1[:])
        inv1 = cpool.tile([1, 1], F32)
        nc.vector.reciprocal(inv1[:], ttemp[:])
        lbs_row = cpool.tile([1, E], F32)
        nc.vector.tensor_scalar_mul(lbs_row[:], lbrow[:], inv1[:])

        # replicate bias*scale to all 128 partitions, and 1/temp to [128,1]
        lb_ps = ps_t.tile([128, E], F32, tag="pst")
        nc.tensor.matmul(lb_ps[:], ones_row[0:1, :], lbs_row[:], start=True, stop=True)
        lbs = cpool.tile([128, E], F32)
        nc.vector.tensor_copy(lbs[:], lb_ps[:])
        iv_ps = ps_t.tile([128, 1], F32, tag="pst")
        nc.tensor.matmul(iv_ps[:], ones_row[0:1, :], inv1[:], start=True, stop=True)
        s128 = cpool.tile([128, 1], F32)
        nc.vector.tensor_copy(s128[:], iv_ps[:])

        # global accumulators
        pacc = accpool.tile([128, E], F32)
        nc.vector.memset(pacc[:], 0.0)
        eacc = accpool.tile([128, 1], F32)
        nc.vector.memset(eacc[:], 0.0)

        # ---- main loop ------------------------------------------------
        for ps, tp in enumerate(PASS_TOK):
            blks = tp // 128
            hil_v = hts[ps]
            acc = ps_acc.tile([128, tp], F32, tag=f"acc{ps}")
            for g in range(HC // PAIR):
                hil = hipool.tile([128, 2 * PAIR * tp], BF16, tag="hil")
                nc.sync.dma_start(hil[:], hil_v[g])
                for par in range(PAIR):
                    h = PAIR * g + par
                    # hi chunk at par*tp, lo chunk at (PAIR+par)*tp
                    for base, first, last in ((par, True, False),
                                              (PAIR + par, False, True)):
                        for half in range(tp // 512):
                            nc.tensor.matmul(
                                acc[:, half * 512:(half + 1) * 512],
                                wp[:, h, :],
                                hil[:, base * tp + half * 512:
                                    base * tp + (half + 1) * 512],
                                start=(h == 0 and first),
                                stop=(h == HC - 1 and last),
                            )

            # PSUM -> SBUF (ACT) so the PE can transpose it
            accsb = acsbpool.tile([128, tp], F32, tag="accsb")
            nc.scalar.copy(accsb[:], acc[:])

            # token-major: transpose 128x128 blocks; cols 0:E = wh part,
            # E:2E = wl part; fold + bias + scale into tm per block
            tm = tmpool.tile([128, blks, E], F32, tag="tm")
            for b in range(blks):
                tp_ps = ps_t.tile([128, 128], F32, tag="pst")
                nc.tensor.transpose(
                    tp_ps[:], accsb[:, b * 128:(b + 1) * 128], ident[:, :]
                )
                # fold halves + bias + scale via two chained STTs, each
                # reading one PSUM half: t = wl_part/temp + lb/temp, then
                # tm = wh_part/temp + t  (only one PSUM input per op)
                fold = spool.tile([128, E], F32, tag="fold")
                nc.vector.scalar_tensor_tensor(
                    fold[:], tp_ps[:, E:2 * E], s128[:], lbs[:],
                    op0=ALU.mult, op1=ALU.add,
                )
                nc.vector.scalar_tensor_tensor(
                    tm[:, b, 0:E], tp_ps[:, 0:E], s128[:], fold[:],
                    op0=ALU.mult, op1=ALU.add,
                )

            # logits stream out while the softmax/top-k chain runs
            o0 = PASS_OFF[ps]
            nc.sync.dma_start(
                out0[o0:o0 + tp, :].rearrange("(j q) c -> q j c", q=128), tm[:]
            )

            lg = tm[:, :, :]
            wi = tmpool.tile([128, blks, 4], F32, tag="wi")

            # softmax over experts
            nmax = spool.tile([128, blks], F32, tag="nmax")
            nc.vector.reduce_max(nmax[:], lg, axis=AX.X, negate=True)
            sh = spool.tile([128, blks, E], F32, tag="sh")
            nc.vector.tensor_tensor(
                sh[:], lg,
                nmax[:].rearrange("q (a o) -> q a o", o=1).broadcast_to((128, blks, E)),
                op=ALU.add,
            )
            ex = spool.tile([128, blks, E], F32, tag="ex")
            nc.scalar.activation(ex[:], sh[:], AF.Exp, bias=zero128[:])

            # top-2 per token (independent of the probs chain; its Exp is
            # issued next to the softmax Exp to avoid an ACT table swap)
            mx = spool.tile([128, blks, 8], F32, tag="mx")
            ix = spool.tile([128, blks, 8], U32, tag="ix")
            for b in range(blks):
                nc.vector.max(mx[:, b, :], tm[:, b, 0:E])
                nc.vector.max_index(ix[:, b, :], mx[:, b, :], tm[:, b, 0:E])
            d2 = spool.tile([128, blks, TOPK], F32, tag="d2")
            nc.vector.tensor_tensor(
                d2[:], mx[:, :, 0:TOPK],
                mx[:, :, 0:1].broadcast_to((128, blks, TOPK)),
                op=ALU.subtract,
            )
            e2 = spool.tile([128, blks, TOPK], F32, tag="e2")
            nc.scalar.activation(e2[:], d2[:], AF.Exp, bias=zero128[:])
            # tiny Ln on a slice of e2 preloads the ACT Ln table while the
            # DVE computes sums/recips — keeps the 1.3us table load off the
            # serial tail chain (reading e2 pins it after the Exp above)
            dummy_ln = spool.tile([1, 1], F32, tag="dummy")
            nc.scalar.activation(dummy_ln[:], e2[0:1, 0, 0:1], AF.Ln, bias=eps1[:])

            ssum = spool.tile([128, blks], F32, tag="ssum")
            nc.vector.reduce_sum(ssum[:], ex[:], axis=AX.X)
            rs = spool.tile([128, blks], F32, tag="rs")
            nc.vector.reciprocal(rs[:], ssum[:])
            pr = spool.tile([128, blks, E], F32, tag="pr")
            nc.vector.tensor_tensor(
                pr[:], ex[:],
                rs[:].rearrange("q (a o) -> q a o", o=1).broadcast_to((128, blks, E)),
                op=ALU.mult,
            )

            # entropy partial: sum over experts and blocks of p*log(p+eps)
            lp = spool.tile([128, blks, E], F32, tag="lp")
            nc.scalar.activation(lp[:], pr[:], AF.Ln, bias=eps128[:])
            pl = spool.tile([128, blks, E], F32, tag="pl")
            nc.vector.tensor_mul(pl[:], pr[:], lp[:])
            entp = spool.tile([128, 1], F32, tag="entp")
            nc.vector.reduce_sum(entp[:], pl[:], axis=AX.XY)
            nc.vector.tensor_add(eacc[:], eacc[:], entp[:])

            # expert-load partial: pairwise-tree sum of probs over blocks
            cur, w_ = pr, blks
            while w_ > 1:
                half = w_ // 2
                nxt = spool.tile([128, half, E], F32, tag=f"tree{half}_{ps}")
                nc.vector.tensor_add(nxt[:], cur[:, 0:half, :], cur[:, half:2 * half, :])
                if w_ % 2:
                    nc.vector.tensor_add(
                        nxt[:, 0:1, :], nxt[:, 0:1, :], cur[:, 2 * half:w_, :]
                    )
                cur, w_ = nxt, half
            nc.vector.tensor_add(pacc[:], pacc[:], cur[:, 0, :])

            s2 = spool.tile([128, blks], F32, tag="s2")
            nc.vector.reduce_sum(s2[:], e2[:], axis=AX.X)
            r2 = spool.tile([128, blks], F32, tag="r2")
            nc.vector.reciprocal(r2[:], s2[:])
            nc.vector.tensor_tensor(
                wi[:, :, 0:TOPK], e2[:],
                r2[:].rearrange("q (a o) -> q a o", o=1).broadcast_to((128, blks, TOPK)),
                op=ALU.mult,
            )
            # indices (uint32 -> f32 convert; values <= 63 are exact)
            nc.vector.tensor_copy(wi[:, :, TOPK:2 * TOPK], ix[:, :, 0:TOPK])

            nc.sync.dma_start(
                out1[o0:o0 + tp, :].rearrange("(q j) c -> q j c", j=blks), wi[:]
            )

        nc.sync.dma_start(pacc_d[:, :], pacc[:])
        nc.sync.dma_start(eacc_d[:, :], eacc[:])

    nc.compile()
    return nc


_NC_CACHE = None


def _get_nc():
    global _NC_CACHE
    if _NC_CACHE is None:
        _NC_CACHE = build_nc()
    return _NC_CACHE


def _split_bf16(x):
    import ml_dtypes
    hi = x.astype(ml_dtypes.bfloat16)
    lo = (x - hi.astype(np.float32)).astype(ml_dtypes.bfloat16)
    return hi, lo


def _pair_layout(hi, lo):
    """Merge hi/lo [H, tp] into [(HC/PAIR)*128, 2*PAIR*tp] DMA blocks."""
    tp = hi.shape[1]
    h4 = hi.reshape(HC // PAIR, PAIR, 128, tp)
    l4 = lo.reshape(HC // PAIR, PAIR, 128, tp)
    return np.ascontiguousarray(
        np.concatenate([h4, l4], axis=1)        # [G, 2*PAIR, 128, tp]
        .transpose(0, 2, 1, 3)                  # [G, 128, 2*PAIR, tp]
        .reshape(HC // PAIR * 128, 2 * PAIR * tp)
    )


def make_in_maps(hidden_states, router_weight, expert_importance, temperature):
    hs = np.ascontiguousarray(np.asarray(hidden_states, dtype=np.float32))
    # [E, H] -> [H, E] -> [HC, 128, E] -> [128, HC, E]
    wt = (
        np.asarray(router_weight, dtype=np.float32).T
        .reshape(HC, 128, E).transpose(1, 0, 2)
    )
    wth, wtl = _split_bf16(np.ascontiguousarray(wt))
    # pack [wh | wl] along the last axis -> [128, HC, 2E] -> [128, HC*2E]
    wtp = np.ascontiguousarray(
        np.concatenate([wth, wtl], axis=2).reshape(128, HC * 2 * E)
    )
    imp = np.asarray(expert_importance, dtype=np.float32).reshape(1, E)
    tmp = np.asarray(temperature, dtype=np.float32).reshape(1, 1)
    in_maps = []
    for c in range(NCORES):
        sh = hs[c * NT:(c + 1) * NT].T  # [H, NT]
        m = {"wtp": wtp, "imp": imp, "temp": tmp}
        for p, tp in enumerate(PASS_TOK):
            o = PASS_OFF[p]
            hi, lo = _split_bf16(np.ascontiguousarray(sh[:, o:o + tp]))
            m[f"hil{p}"] = _pair_layout(hi, lo)
        in_maps.append(m)
    return in_maps


def postprocess(results):
    logits = np.empty((N, E), np.float32)
    idx = np.empty((N, TOPK), np.int32)
    ew = np.empty((N, TOPK), np.float32)
    load_sum = np.zeros(E, np.float64)
    ent_sum = 0.0
    for c, r in enumerate(results):
        logits[c * NT:(c + 1) * NT] = r["out0"]
        # out1 rows are (partition, block)-ordered within each pass range
        wi = np.empty((NT, 4), np.float32)
        for p, tp in enumerate(PASS_TOK):
            o = PASS_OFF[p]
            blks = tp // 128
            wi[o:o + tp] = (
                r["out1"][o:o + tp].reshape(128, blks, 4)
                .transpose(1, 0, 2).reshape(tp, 4)
            )
        ew[c * NT:(c + 1) * NT] = wi[:, 0:TOPK]
        idx[c * NT:(c + 1) * NT] = np.rint(wi[:, TOPK:2 * TOPK]).astype(np.int32)
        load_sum += r["pacc"].astype(np.float64).sum(axis=0)
        ent_sum += float(r["eacc"].astype(np.float64).sum())
    expert_load = (load_sum / N).astype(np.float32)
    load_var = np.float32(np.var(load_sum / N, ddof=1))
    entropy = np.float32(-ent_sum / N)
    return (logits, idx, ew, expert_load, load_var, entropy)


def kernel(hidden_states, router_weight, expert_importance, temperature, top_k):
    assert int(top_k) == TOPK
    nc = _get_nc()
    in_maps = make_in_maps(hidden_states, router_weight, expert_importance, temperature)
    res = run_bass_kernel_spmd(nc, in_maps, core_ids=list(range(NCORES)))
    return postprocess(res.results)
